# revision 20
# baseline (speedup 1.0000x reference)
"""MoE layer (top-k routing) on 8 Trainium2 NeuronCores.

Expert-parallel per the sharding hint: the host computes router softmax +
top-k (0.1% of FLOPs) and realizes the "all-to-all dispatch by expert
assignment" while building the per-core SPMD input maps; each core runs
expert FFN work (fp32 PSUM accumulation); the host applies the combine
weights and scatter-adds results back to [B,N,C].

Load balance: each expert's FFN is split along D_FF into four quarter-units
(exact: gelu is elementwise over F and GEMM2 contracts F, so the four
partial y's just add). The 32 quarter-units are assigned four per core, one
per slot class A-D: slot A holds the two largest experts' quarters, slot B
the next two, etc. Each slot is padded to the max count within its pair, so
per-core padded work is sum over slots of max(pair) — within ~1% of the
perfect-balance floor — instead of 2*max(all counts).

Mixed precision: the PE array is the bottleneck (96% busy at the bf16
streaming roofline), and fp8e4 DoubleRow matmuls run ~1.5-1.8x faster per
MAC. Full-fp8 would put the output at ~5.5e-2 rel err (gate is 2e-2), but
the error contribution of a token-expert pair scales with its combine
weight, so each expert's lowest-weight FRAC8 tokens run through an fp8
pipeline (both GEMMs, e4m3 operands, weights pre-scaled x128 into e4m3's
normal range, epilogues rescale via the activation's scale operand) and the
rest stay bf16. Quantization noise ~ 5.5e-2 * sqrt(sum w^2 over fp8 pairs /
total), tuned to keep total rel err < 1.5e-2.
"""

import json
import os
import sys
import types

import numpy as np
import ml_dtypes

D_MODEL = 1024
D_FF = 4096
N_EXPERTS = 8
N_CORES = 8

P = 128
CB = D_MODEL // P      # 8 c-blocks of 128
FQ = D_FF // 4         # F quarter = 1024
FBQ = FQ // P          # 8 f-blocks per quarter
TN = 512               # token tile (matmul moving free dim / one PSUM bank)
SLOTS = ("A", "B", "C", "D")

# fraction of each expert's tokens (lowest combine weight first) on the fp8
# pipeline; weights are pre-scaled by SW8 so xavier-range values land in
# e4m3's normal range (epilogue rescales by 1/SW8)
FRAC8 = float(os.environ.get("BASS_MOE_FRAC8", "0.22"))
SW8 = 128.0


def _shim_axon_hooks():
    """Register the NTFF profile hook bass_utils looks for under axon; the
    image's `antenv` stub lacks `axon_hooks`."""
    if "antenv.axon_hooks" in sys.modules:
        return
    try:
        import trn_agent_boot.trn_boot as _tb
        hook = _tb._ntff_profile_via_ctypes("/opt/axon/libaxon_pjrt.so")
    except Exception:
        hook = None
    mod = types.ModuleType("antenv.axon_hooks")
    mod.get_axon_ntff_profile_hook = lambda: hook
    mod.set_axon_ntff_profile_hook = lambda h: None
    sys.modules["antenv.axon_hooks"] = mod


_shim_axon_hooks()

import concourse.bass as bass            # noqa: E402
import concourse.tile as tile            # noqa: E402
from concourse import mybir              # noqa: E402
from concourse.bass import ds, ts        # noqa: E402
from concourse.bass_utils import run_bass_kernel_spmd  # noqa: E402


def _fix_multiwait_bir(nc):
    """Split instructions carrying >1 sync wait (the TileContext tail drain)
    into single-wait NoOps; this walrus build rejects multi-wait CTRL
    instructions."""
    raw = bass.Bass.to_json_bytes(nc)
    d = json.loads(raw)
    for f in d["functions"]:
        for b in f["blocks"]:
            out = []
            for i in b["instructions"]:
                si = i.get("sync_info") or {}
                waits = si.get("on_wait") or []
                if len(waits) > 1:
                    for k, w in enumerate(waits[:-1]):
                        out.append({
                            "name": f"{i['name']}_wsplit{k}",
                            "engine": i["engine"],
                            "ins": [], "outs": [],
                            "opcode": "NoOp",
                            "sync_info": {"on_update": [], "on_wait": [w]},
                        })
                    si["on_wait"] = [waits[-1]]
                out.append(i)
            b["instructions"] = out
    fixed = json.dumps(d).encode()
    nc.to_json_bytes = lambda: fixed


_NC_CACHE = {}


def _token_tiles(cap):
    tiles, off = [], 0
    while off < cap:
        tw = min(TN, cap - off)
        tiles.append((off, tw))
        off += tw
    return tiles


def _build_moe_kernel(caps, caps8):
    """Four quarter-expert FFN units per core (slots A-D), SPMD x8. Each
    unit has a bf16 token segment (cap) and an fp8 token segment (cap8)."""
    key = (tuple(caps), tuple(caps8))
    if key in _NC_CACHE:
        return _NC_CACHE[key]

    bf16 = mybir.dt.bfloat16
    f32 = mybir.dt.float32
    f8 = mybir.dt.float8e4
    Act = mybir.ActivationFunctionType
    DR = mybir.MatmulPerfMode.DoubleRow

    nc = bass.Bass("TRN2", target_bir_lowering=False, debug=False,
                   num_devices=N_CORES)

    # all slots' biases pre-swizzled by the host into one partition-major
    # param: [p, slot*16 + g] = b1_q[g*128+p], [p, slot*16+8+g] = b2[g*128+p].
    # The naive per-slot (g p) -> p g bias DMA is 1024 scattered 4-byte
    # descriptors; at program start it blocked the scalar DGE queue ~18us.
    biasv = nc.declare_dram_parameter("biasv", [P * 16 * len(SLOTS)], f32,
                                      isOutput=False)
    biasr = biasv.ap().rearrange("(p g) -> p g", p=P)   # [128, 64]

    # all bulk tensors are partition-major on the host so every DMA
    # descriptor covers one partition's full contiguous span (8-16KB).
    # The DGE issues ~60 descriptors/us/queue, so descriptor size sets DMA
    # throughput: the old (g p)-major layouts fragmented into 1-2KB chunks
    # and crawled at ~55GB/s, starving the PE at startup. Token streams are
    # additionally pre-tiled into [ntile, P, CB, TN] (+ ragged remainder).
    units = []
    for slot, cap, cap8 in zip(SLOTS, caps, caps8):
        u = {"cap": cap, "cap8": cap8, "slot": slot}
        nf, rem = cap // TN, cap % TN
        u["nf"], u["rem"] = nf, rem
        u["w1r"] = nc.declare_dram_parameter(f"w1t{slot}", [P, CB, FQ], bf16, isOutput=False).ap()
        u["w2r"] = nc.declare_dram_parameter(f"w2t{slot}", [P, FBQ, D_MODEL], bf16, isOutput=False).ap()
        # partials return as bf16: halves the output DMA so total traffic
        # stays under the chip's P0 power-throttle trigger (observed: the
        # f32 version pushed PE from 2.4 to 2.0 GHz); host sums in f32
        tws = [TN] * nf + ([rem] if rem else [])
        u["xaps"] = [nc.declare_dram_parameter(f"xT{slot}_{i}", [P, CB, tw], bf16,
                                               isOutput=False).ap()
                     for i, tw in enumerate(tws)]
        u["yaps"] = [nc.declare_dram_parameter(f"yT{slot}_{i}", [P, CB, tw], bf16,
                                               isOutput=True).ap()
                     for i, tw in enumerate(tws)]
        u["tiles"] = tws
        nf8, rem8 = cap8 // TN, cap8 % TN
        u["nf8"], u["rem8"] = nf8, rem8
        tws8 = [TN] * nf8 + ([rem8] if rem8 else [])
        if cap8:
            u["w18r"] = nc.declare_dram_parameter(f"w1t8{slot}", [P, CB, FQ], f8, isOutput=False).ap()
            u["w28r"] = nc.declare_dram_parameter(f"w2t8{slot}", [P, FBQ, D_MODEL], f8, isOutput=False).ap()
            u["x8aps"] = [nc.declare_dram_parameter(f"xT8{slot}_{i}", [P, CB, tw], f8,
                                                    isOutput=False).ap()
                          for i, tw in enumerate(tws8)]
            u["y8aps"] = [nc.declare_dram_parameter(f"yT8{slot}_{i}", [P, CB, tw], bf16,
                                                    isOutput=True).ap()
                          for i, tw in enumerate(tws8)]
        u["tiles8"] = tws8
        units.append(u)

    with tile.TileContext(nc) as tc:
        with (
            tc.tile_pool(name="weights", bufs=1) as wpool,
            tc.tile_pool(name="w8", bufs=1) as w8pool,
            tc.tile_pool(name="xin", bufs=2) as xpool,
            tc.tile_pool(name="x8in", bufs=2) as x8pool,
            tc.tile_pool(name="hbuf", bufs=1) as hpool,
            tc.tile_pool(name="h8buf", bufs=1) as h8pool,
            tc.tile_pool(name="yout", bufs=2) as ypool,
            tc.tile_pool(name="psum", bufs=4, space="PSUM") as psum,
        ):
            # ---- loads, spread over three independent HW-DGE queues so x
            # tiles and y stores never queue behind weight bulk (a single
            # queue stalls the PE when a tile's x sits behind 12.6MB of
            # unit B-D weights):
            #   sync   — x tiles + y stores only (streaming traffic)
            #   scalar — unit A: b1, b2, then w1 in two halves (low half
            #            first so GEMM1 m-blocks 0-3 start at x0-arrival)
            #   gpsimd — unit A w2 first, then unit B-D biases + weights,
            #            then per-unit fp8 weight copies (issued just-in-
            #            time from the compute loop)
            # startup critical path: x0 split across sync+gpsimd halves,
            # w1A in quarter-DMAs so GEMM1 m-blocks start on the first
            # quarter's arrival (~12us) instead of the full-w1 DMA (~25us)
            ua = units[0]
            ua["x0"] = xpool.tile([P, CB, TN], bf16, tag="xt", name="x0A")
            nc.sync.dma_start(ua["x0"][:, 0:4, :], ua["xaps"][0][:, 0:4, :])
            nc.gpsimd.dma_start(ua["x0"][:, 4:CB, :], ua["xaps"][0][:, 4:CB, :])

            ua["w1_sb"] = wpool.tile([P, CB, FQ], bf16, tag="w1A", name="w1A")
            nc.scalar.dma_start(ua["w1_sb"][:, :, 0:256], ua["w1r"][:, :, 0:256])
            bias_sb = wpool.tile([P, 16 * len(SLOTS)], f32, tag="biasv",
                                 name="biasv")
            nc.scalar.dma_start(bias_sb[:], biasr)
            for q0 in range(256, FQ, 256):
                nc.scalar.dma_start(ua["w1_sb"][:, :, q0:q0 + 256],
                                    ua["w1r"][:, :, q0:q0 + 256])

            ua["w2_sb"] = wpool.tile([P, FBQ, D_MODEL], bf16, tag="w2A", name="w2A")
            nc.gpsimd.dma_start(ua["w2_sb"][:, :, :], ua["w2r"][:, :, :])

            # unit A tile 1 tokens right behind tile 0 on the x queue
            if ua["nf"] > 1:
                ua["x1"] = xpool.tile([P, CB, TN], bf16, tag="xt", name="x1A")
                nc.sync.dma_start(ua["x1"][:, :, :], ua["xaps"][1][:, :, :])

            for u in units[1:]:
                slot = u["slot"]
                u["w1_sb"] = wpool.tile([P, CB, FQ], bf16, tag=f"w1{slot}",
                                        name=f"w1{slot}")
                nc.gpsimd.dma_start(u["w1_sb"][:, :, :], u["w1r"][:, :, :])
                u["w2_sb"] = wpool.tile([P, FBQ, D_MODEL], bf16, tag=f"w2{slot}",
                                        name=f"w2{slot}")
                nc.gpsimd.dma_start(u["w2_sb"][:, :, :], u["w2r"][:, :, :])

            for s, u in enumerate(units):
                u["b1o"], u["b2o"] = s * 16, s * 16 + 8

            # ---- compute: per unit, bf16 token tiles then fp8 token tiles
            nu = len(units)
            for ui, u in enumerate(units):
                if u["cap8"]:
                    # fp8 weights land in a single shared buffer; the DMA
                    # WAR-waits on the previous unit's fp8 matmuls and runs
                    # during this unit's bf16 phase
                    u["w18_sb"] = w8pool.tile([P, CB, FQ], f8, tag="w18", name=f"w18{u['slot']}")
                    nc.gpsimd.dma_start(u["w18_sb"][:, :, :], u["w18r"][:, :, :])
                    u["w28_sb"] = w8pool.tile([P, FBQ, D_MODEL], f8, tag="w28", name=f"w28{u['slot']}")
                    nc.gpsimd.dma_start(u["w28_sb"][:, :, :], u["w28r"][:, :, :])

                for ti, tw in enumerate(u["tiles"]):
                    if ti == 0 and "x0" in u:
                        xt = u["x0"]
                    elif ti == 1 and "x1" in u:
                        xt = u["x1"]
                    else:
                        xt = xpool.tile([P, CB, TN], bf16, tag="xt")
                        nc.sync.dma_start(xt[:, :, :tw], u["xaps"][ti][:, :, :])

                    ht = hpool.tile([P, FBQ, TN], bf16, tag="ht")
                    for m in range(FBQ):
                        ph = psum.tile([P, TN], f32, tag="ph")
                        for k in range(CB):
                            nc.tensor.matmul(
                                ph[:, :tw],
                                lhsT=u["w1_sb"][:, k, ts(m, P)],
                                rhs=xt[:, k, :tw],
                                start=(k == 0), stop=(k == CB - 1),
                            )
                        nc.scalar.activation(ht[:, m, :tw], ph[:, :tw], Act.Gelu,
                                             bias=bias_sb[:, u["b1o"] + m:u["b1o"] + m + 1])

                    yt = ypool.tile([P, CB, TN], bf16, tag="yt")
                    for c in range(CB):
                        py = psum.tile([P, TN], f32, tag="py")
                        for k in range(FBQ):
                            nc.tensor.matmul(
                                py[:, :tw],
                                lhsT=u["w2_sb"][:, k, ts(c, P)],
                                rhs=ht[:, k, :tw],
                                start=(k == 0), stop=(k == FBQ - 1),
                            )
                        nc.scalar.add(yt[:, c, :tw], py[:, :tw],
                                      bias_sb[:, u["b2o"] + c:u["b2o"] + c + 1])
                    nc.sync.dma_start(u["yaps"][ti][:, :, :], yt[:, :, :tw])

                for ti, tw in enumerate(u["tiles8"]):
                    x8t = x8pool.tile([P, CB, TN], f8, tag="x8")
                    nc.scalar.dma_start(x8t[:, :, :tw], u["x8aps"][ti][:, :, :])

                    h8t = h8pool.tile([P, FBQ, TN], f8, tag="h8")
                    for m in range(FBQ):
                        ph = psum.tile([P, TN], f32, tag="ph")
                        for j in range(CB // 2):
                            nc.tensor.matmul(
                                ph[:, :tw],
                                lhsT=u["w18_sb"][:, 2 * j:2 * j + 2, ts(m, P)],
                                rhs=x8t[:, 2 * j:2 * j + 2, :tw],
                                start=(j == 0), stop=(j == CB // 2 - 1),
                                perf_mode=DR,
                            )
                        # psum holds SW8 * z; gelu(psum/SW8 + b1) -> e4m3 h
                        nc.scalar.activation(h8t[:, m, :tw], ph[:, :tw], Act.Gelu,
                                             bias=bias_sb[:, u["b1o"] + m:u["b1o"] + m + 1],
                                             scale=1.0 / SW8)

                    last = (ui == nu - 1) and (ti == len(u["tiles8"]) - 1)
                    yt = ypool.tile([P, CB, TN], bf16, tag="yt")
                    for c in range(CB):
                        py = psum.tile([P, TN], f32, tag="py")
                        for j in range(FBQ // 2):
                            nc.tensor.matmul(
                                py[:, :tw],
                                lhsT=u["w28_sb"][:, 2 * j:2 * j + 2, ts(c, P)],
                                rhs=h8t[:, 2 * j:2 * j + 2, :tw],
                                start=(j == 0), stop=(j == FBQ // 2 - 1),
                                perf_mode=DR,
                            )
                        nc.scalar.activation(yt[:, c, :tw], py[:, :tw], Act.Identity,
                                             bias=bias_sb[:, u["b2o"] + c:u["b2o"] + c + 1],
                                             scale=1.0 / SW8)
                        if last:
                            # final tile: per-block stores overlap the tail
                            # GEMM2 blocks instead of one post-loop DMA
                            nc.sync.dma_start(u["y8aps"][ti][:, c, :tw],
                                              yt[:, c, :tw])
                    if not last:
                        nc.sync.dma_start(u["y8aps"][ti][:, :, :], yt[:, :, :tw])

    _fix_multiwait_bir(nc)
    _NC_CACHE[key] = nc
    return nc


def _route(xf, router_w, k):
    """Replicate the reference router numerics (f32 softmax, top-k, renorm)."""
    logits = xf @ router_w.T.astype(np.float32)          # [T, E]
    m = logits.max(axis=-1, keepdims=True)
    e = np.exp(logits - m, dtype=np.float32)
    probs = e / e.sum(axis=-1, keepdims=True)
    # descending, ties -> lower index first (matches jax.lax.top_k)
    idx = np.argsort(-probs, axis=-1, kind="stable")[:, :k]   # [T, k]
    w = np.take_along_axis(probs, idx, axis=-1)               # [T, k]
    w = w / (w.sum(axis=-1, keepdims=True) + 1e-9)
    return idx, w


def _align16(n):
    return max(P, -(-n // 16) * 16)


def _align16s(n):
    return max(16, -(-n // 16) * 16) if n else 0


def _pmaj(a2d, blocks):
    """[(g p), f] row-major 2D -> partition-major [P, blocks, f]."""
    return np.ascontiguousarray(
        a2d.reshape(blocks, P, a2d.shape[1]).transpose(1, 0, 2))


def _tile_tok(a2d, cap):
    """[C, n] token stream (n <= cap) -> list of partition-major per-tile
    arrays [P, CB, tw] (tw = TN for full tiles + remainder), zero-padded."""
    C = a2d.shape[0]
    pad = np.zeros((C, cap), dtype=a2d.dtype)
    pad[:, :a2d.shape[1]] = a2d
    tmp = pad.reshape(CB, P, cap)
    tiles, off = [], 0
    while off < cap:
        tw = min(TN, cap - off)
        tiles.append(np.ascontiguousarray(
            tmp[:, :, off:off + tw].transpose(1, 0, 2)))
        off += tw
    return tiles


def _untile_tok(tiles, cap):
    """inverse of _tile_tok -> [C, cap] f32."""
    parts = [t.astype(np.float32).transpose(1, 0, 2) for t in tiles]
    return np.concatenate(parts, axis=2).reshape(CB * P, cap)


def _q8(a, scale=1.0):
    a = np.asarray(a, np.float32) * np.float32(scale)
    np.clip(a, -240.0, 240.0, out=a)
    return a.astype(ml_dtypes.float8_e4m3)


def kernel(x, router_w, expert_w1, expert_b1, expert_w2, expert_b2, top_k):
    x = np.asarray(x)
    router_w = np.asarray(router_w, dtype=np.float32)
    expert_w1 = np.asarray(expert_w1, dtype=np.float32)
    expert_b1 = np.asarray(expert_b1, dtype=np.float32)
    expert_w2 = np.asarray(expert_w2, dtype=np.float32)
    expert_b2 = np.asarray(expert_b2, dtype=np.float32)
    k = int(np.asarray(top_k))
    Bq, Nq, C = x.shape
    Tq = Bq * Nq
    E = expert_w1.shape[0]
    xf = np.ascontiguousarray(x.reshape(Tq, C), dtype=np.float32)

    idx, w = _route(xf, router_w, k)

    # per expert: token list sorted by ascending combine weight, so the
    # lowest-weight FRAC8 go on the fp8 pipeline (their quantization noise
    # is scaled by w in the combine)
    tok8_idx, tok8_w, tokb_idx, tokb_w = [], [], [], []
    for e in range(E):
        mask = idx == e
        sel = np.nonzero(mask.any(axis=-1))[0]
        we = (w * mask).sum(axis=-1)[sel].astype(np.float32)
        order_w = np.argsort(we, kind="stable")
        n8 = int(round(FRAC8 * len(sel)))
        tok8_idx.append(sel[order_w[:n8]])
        tok8_w.append(we[order_w[:n8]])
        tokb_idx.append(sel[order_w[n8:]])
        tokb_w.append(we[order_w[n8:]])
    counts = np.array([len(s) for s in tokb_idx])
    counts8 = np.array([len(s) for s in tok8_idx])

    # slot s holds the quarters of the experts ranked 2s and 2s+1 by total
    # count; cores 0-3 take quarters 0-3 of the first, cores 4-7 the second
    order = np.argsort(-(counts + counts8), kind="stable")
    caps = [_align16(int(max(counts[order[2 * s]], counts[order[2 * s + 1]])))
            for s in range(4)]
    caps8 = [_align16s(int(max(counts8[order[2 * s]], counts8[order[2 * s + 1]])))
             for s in range(4)]

    nc = _build_moe_kernel(caps, caps8)

    # one xT (+xT8) per expert, shared by its four quarter-units
    xTs, xT8s = {}, {}
    for s in range(4):
        for j in (0, 1):
            e = int(order[2 * s + j])
            xTs[e] = _tile_tok(xf[tokb_idx[e]].T.astype(ml_dtypes.bfloat16),
                               caps[s])
            if caps8[s]:
                xT8s[e] = _tile_tok(_q8(xf[tok8_idx[e]].T), caps8[s])

    in_maps = [dict() for _ in range(N_CORES)]
    biasvs = [np.zeros((P, 16 * len(SLOTS)), dtype=np.float32)
              for _ in range(N_CORES)]
    placement = {}          # (expert, quarter) -> (core, slot name)
    for s, slot in enumerate(SLOTS):
        for core in range(N_CORES):
            e = int(order[2 * s + (0 if core < 4 else 1)])
            q = core % 4
            placement[(e, q)] = (core, slot)
            f0, f1 = q * FQ, (q + 1) * FQ
            b2 = expert_b2[e] if q == 0 else np.zeros(C, dtype=np.float32)
            w1q = np.ascontiguousarray(expert_w1[e, f0:f1].T)
            w2q = np.ascontiguousarray(expert_w2[e, :, f0:f1].T)
            bv = biasvs[core]
            bv[:, s * 16:s * 16 + 8] = expert_b1[e, f0:f1].reshape(8, P).T
            bv[:, s * 16 + 8:s * 16 + 16] = b2.reshape(8, P).T
            in_maps[core].update({
                f"w1t{slot}": _pmaj(w1q.astype(ml_dtypes.bfloat16), CB),
                f"w2t{slot}": _pmaj(w2q.astype(ml_dtypes.bfloat16), FBQ),
            })
            for i, tarr in enumerate(xTs[e]):
                in_maps[core][f"xT{slot}_{i}"] = tarr
            if caps8[s]:
                in_maps[core].update({
                    f"w1t8{slot}": _pmaj(_q8(w1q, SW8), CB),
                    f"w2t8{slot}": _pmaj(_q8(w2q, SW8), FBQ),
                })
                for i, tarr in enumerate(xT8s[e]):
                    in_maps[core][f"xT8{slot}_{i}"] = tarr

    for core in range(N_CORES):
        in_maps[core]["biasv"] = biasvs[core].reshape(-1)

    trace = os.environ.get("BASS_MOE_TRACE") == "1"
    res = run_bass_kernel_spmd(
        nc, in_maps, core_ids=list(range(N_CORES)),
        trace=trace,
        tmpdir=os.environ.get("BASS_MOE_TMPDIR") if trace else None,
    )
    if trace:
        kernel.last_exec_time_ns = res.exec_time_ns
        kernel.last_trace = (res.instructions_and_trace or (None, None))[1]

    out = np.zeros((Tq, C), dtype=np.float32)
    for e in range(E):
        for tok, tw_, pfx, cnt, capv in (
                (tokb_idx[e], tokb_w[e], "yT", counts[e], None),
                (tok8_idx[e], tok8_w[e], "yT8", counts8[e], None)):
            if not cnt:
                continue
            acc = np.zeros((cnt, C), dtype=np.float32)
            for q in range(4):
                core, slot = placement[(e, q)]
                s = SLOTS.index(slot)
                capv = caps[s] if pfx == "yT" else caps8[s]
                r = res.results[core]
                tlist = []
                i = 0
                while f"{pfx}{slot}_{i}" in r:
                    tlist.append(r[f"{pfx}{slot}_{i}"])
                    i += 1
                acc += _untile_tok(tlist, capv)[:, :cnt].T
            out[tok] += acc * tw_[:, None]
    return out.reshape(Bq, Nq, C).astype(x.dtype)


# revision 21
# speedup vs baseline: 1.0411x; 1.0411x over previous
"""MoE layer (top-k routing) on 8 Trainium2 NeuronCores.

Expert-parallel per the sharding hint: the host computes router softmax +
top-k (0.1% of FLOPs) and realizes the "all-to-all dispatch by expert
assignment" while building the per-core SPMD input maps; each core runs
expert FFN work (fp32 PSUM accumulation); the host applies the combine
weights and scatter-adds results back to [B,N,C].

Load balance: each expert's FFN is split along D_FF into four quarter-units
(exact: gelu is elementwise over F and GEMM2 contracts F, so the four
partial y's just add). The 32 quarter-units are assigned four per core, one
per slot class A-D: slot A holds the two largest experts' quarters, slot B
the next two, etc. Each slot is padded to the max count within its pair, so
per-core padded work is sum over slots of max(pair) — within ~1% of the
perfect-balance floor — instead of 2*max(all counts).

Mixed precision: the PE array is the bottleneck (96% busy at the bf16
streaming roofline), and fp8e4 DoubleRow matmuls run ~1.5-1.8x faster per
MAC. Full-fp8 would put the output at ~5.5e-2 rel err (gate is 2e-2), but
the error contribution of a token-expert pair scales with its combine
weight, so each expert's lowest-weight FRAC8 tokens run through an fp8
pipeline (both GEMMs, e4m3 operands, weights pre-scaled x128 into e4m3's
normal range, epilogues rescale via the activation's scale operand) and the
rest stay bf16. Quantization noise ~ 5.5e-2 * sqrt(sum w^2 over fp8 pairs /
total), tuned to keep total rel err < 1.5e-2.
"""

import json
import os
import sys
import types

import numpy as np
import ml_dtypes

D_MODEL = 1024
D_FF = 4096
N_EXPERTS = 8
N_CORES = 8

P = 128
CB = D_MODEL // P      # 8 c-blocks of 128
FQ = D_FF // 4         # F quarter = 1024
FBQ = FQ // P          # 8 f-blocks per quarter
TN = 512               # token tile (matmul moving free dim / one PSUM bank)
SLOTS = ("A", "B", "C", "D")

# fraction of each expert's tokens (lowest combine weight first) on the fp8
# pipeline; weights are pre-scaled by SW8 so xavier-range values land in
# e4m3's normal range (epilogue rescales by 1/SW8)
FRAC8 = float(os.environ.get("BASS_MOE_FRAC8", "0.22"))
SW8 = 128.0


def _shim_axon_hooks():
    """Register the NTFF profile hook bass_utils looks for under axon; the
    image's `antenv` stub lacks `axon_hooks`."""
    if "antenv.axon_hooks" in sys.modules:
        return
    try:
        import trn_agent_boot.trn_boot as _tb
        hook = _tb._ntff_profile_via_ctypes("/opt/axon/libaxon_pjrt.so")
    except Exception:
        hook = None
    mod = types.ModuleType("antenv.axon_hooks")
    mod.get_axon_ntff_profile_hook = lambda: hook
    mod.set_axon_ntff_profile_hook = lambda h: None
    sys.modules["antenv.axon_hooks"] = mod


_shim_axon_hooks()

import concourse.bass as bass            # noqa: E402
import concourse.tile as tile            # noqa: E402
from concourse import mybir              # noqa: E402
from concourse.bass import ds, ts        # noqa: E402
from concourse.bass_utils import run_bass_kernel_spmd  # noqa: E402


def _fix_multiwait_bir(nc):
    """Split instructions carrying >1 sync wait (the TileContext tail drain)
    into single-wait NoOps; this walrus build rejects multi-wait CTRL
    instructions."""
    raw = bass.Bass.to_json_bytes(nc)
    d = json.loads(raw)
    for f in d["functions"]:
        for b in f["blocks"]:
            out = []
            for i in b["instructions"]:
                si = i.get("sync_info") or {}
                waits = si.get("on_wait") or []
                if len(waits) > 1:
                    for k, w in enumerate(waits[:-1]):
                        out.append({
                            "name": f"{i['name']}_wsplit{k}",
                            "engine": i["engine"],
                            "ins": [], "outs": [],
                            "opcode": "NoOp",
                            "sync_info": {"on_update": [], "on_wait": [w]},
                        })
                    si["on_wait"] = [waits[-1]]
                out.append(i)
            b["instructions"] = out
    fixed = json.dumps(d).encode()
    nc.to_json_bytes = lambda: fixed


_NC_CACHE = {}


def _token_tiles(cap):
    tiles, off = [], 0
    while off < cap:
        tw = min(TN, cap - off)
        tiles.append((off, tw))
        off += tw
    return tiles


def _build_moe_kernel(caps, caps8):
    """Four quarter-expert FFN units per core (slots A-D), SPMD x8. Each
    unit has a bf16 token segment (cap) and an fp8 token segment (cap8)."""
    key = (tuple(caps), tuple(caps8))
    if key in _NC_CACHE:
        return _NC_CACHE[key]

    bf16 = mybir.dt.bfloat16
    f32 = mybir.dt.float32
    f8 = mybir.dt.float8e4
    Act = mybir.ActivationFunctionType
    DR = mybir.MatmulPerfMode.DoubleRow

    nc = bass.Bass("TRN2", target_bir_lowering=False, debug=False,
                   num_devices=N_CORES)

    # all slots' biases pre-swizzled by the host into one partition-major
    # param: [p, slot*16 + g] = b1_q[g*128+p], [p, slot*16+8+g] = b2[g*128+p].
    # The naive per-slot (g p) -> p g bias DMA is 1024 scattered 4-byte
    # descriptors; at program start it blocked the scalar DGE queue ~18us.
    biasv = nc.declare_dram_parameter("biasv", [P * 16 * len(SLOTS)], f32,
                                      isOutput=False)
    biasr = biasv.ap().rearrange("(p g) -> p g", p=P)   # [128, 64]

    # all bulk tensors are partition-major on the host so every DMA
    # descriptor covers one partition's full contiguous span (8-16KB).
    # The DGE issues ~60 descriptors/us/queue, so descriptor size sets DMA
    # throughput: the old (g p)-major layouts fragmented into 1-2KB chunks
    # and crawled at ~55GB/s, starving the PE at startup. Token streams are
    # additionally pre-tiled into [ntile, P, CB, TN] (+ ragged remainder).
    units = []
    for slot, cap, cap8 in zip(SLOTS, caps, caps8):
        u = {"cap": cap, "cap8": cap8, "slot": slot}
        nf, rem = cap // TN, cap % TN
        u["nf"], u["rem"] = nf, rem
        if slot == "A":
            u["w1qr"] = [nc.declare_dram_parameter(f"w1t{slot}_q{j}",
                                                   [P, CB, FQ // 4], bf16,
                                                   isOutput=False).ap()
                         for j in range(4)]
        else:
            u["w1r"] = nc.declare_dram_parameter(f"w1t{slot}", [P, CB, FQ], bf16, isOutput=False).ap()
        u["w2r"] = nc.declare_dram_parameter(f"w2t{slot}", [P, FBQ, D_MODEL], bf16, isOutput=False).ap()
        # partials return as bf16: halves the output DMA so total traffic
        # stays under the chip's P0 power-throttle trigger (observed: the
        # f32 version pushed PE from 2.4 to 2.0 GHz); host sums in f32
        tws = [TN] * nf + ([rem] if rem else [])
        u["xaps"] = [nc.declare_dram_parameter(f"xT{slot}_{i}", [P, CB, tw], bf16,
                                               isOutput=False).ap()
                     for i, tw in enumerate(tws)]
        u["yaps"] = [nc.declare_dram_parameter(f"yT{slot}_{i}", [P, CB, tw], bf16,
                                               isOutput=True).ap()
                     for i, tw in enumerate(tws)]
        u["tiles"] = tws
        nf8, rem8 = cap8 // TN, cap8 % TN
        u["nf8"], u["rem8"] = nf8, rem8
        tws8 = [TN] * nf8 + ([rem8] if rem8 else [])
        if cap8:
            u["w18r"] = nc.declare_dram_parameter(f"w1t8{slot}", [P, CB, FQ], f8, isOutput=False).ap()
            u["w28r"] = nc.declare_dram_parameter(f"w2t8{slot}", [P, FBQ, D_MODEL], f8, isOutput=False).ap()
            u["x8aps"] = [nc.declare_dram_parameter(f"xT8{slot}_{i}", [P, CB, tw], f8,
                                                    isOutput=False).ap()
                          for i, tw in enumerate(tws8)]
            u["y8aps"] = [nc.declare_dram_parameter(f"yT8{slot}_{i}", [P, CB, tw], bf16,
                                                    isOutput=True).ap()
                          for i, tw in enumerate(tws8)]
        u["tiles8"] = tws8
        units.append(u)

    with tile.TileContext(nc) as tc:
        with (
            tc.tile_pool(name="weights", bufs=1) as wpool,
            tc.tile_pool(name="w8", bufs=1) as w8pool,
            tc.tile_pool(name="xin", bufs=2) as xpool,
            tc.tile_pool(name="x8in", bufs=2) as x8pool,
            tc.tile_pool(name="hbuf", bufs=1) as hpool,
            tc.tile_pool(name="h8buf", bufs=1) as h8pool,
            tc.tile_pool(name="yout", bufs=2) as ypool,
            tc.tile_pool(name="psum", bufs=4, space="PSUM") as psum,
        ):
            # ---- loads, spread over three independent HW-DGE queues so x
            # tiles and y stores never queue behind weight bulk (a single
            # queue stalls the PE when a tile's x sits behind 12.6MB of
            # unit B-D weights):
            #   sync   — x tiles + y stores only (streaming traffic)
            #   scalar — unit A: b1, b2, then w1 in two halves (low half
            #            first so GEMM1 m-blocks 0-3 start at x0-arrival)
            #   gpsimd — unit A w2 first, then unit B-D biases + weights,
            #            then per-unit fp8 weight copies (issued just-in-
            #            time from the compute loop)
            # startup critical path: x0 split across sync+gpsimd halves,
            # w1A in quarter-DMAs so GEMM1 m-blocks start on the first
            # quarter's arrival (~12us) instead of the full-w1 DMA (~25us)
            ua = units[0]
            ua["x0"] = xpool.tile([P, CB, TN], bf16, tag="xt", name="x0A")
            nc.sync.dma_start(ua["x0"][:, 0:4, :], ua["xaps"][0][:, 0:4, :])
            nc.gpsimd.dma_start(ua["x0"][:, 4:CB, :], ua["xaps"][0][:, 4:CB, :])

            ua["w1_sb"] = wpool.tile([P, 4, CB, FQ // 4], bf16, tag="w1A",
                                     name="w1A")
            nc.scalar.dma_start(ua["w1_sb"][:, 0, :, :], ua["w1qr"][0][:, :, :])
            bias_sb = wpool.tile([P, 16 * len(SLOTS)], f32, tag="biasv",
                                 name="biasv")
            nc.scalar.dma_start(bias_sb[:], biasr)
            for j in range(1, 4):
                nc.scalar.dma_start(ua["w1_sb"][:, j, :, :],
                                    ua["w1qr"][j][:, :, :])

            ua["w2_sb"] = wpool.tile([P, FBQ, D_MODEL], bf16, tag="w2A", name="w2A")
            nc.gpsimd.dma_start(ua["w2_sb"][:, :, :], ua["w2r"][:, :, :])

            # unit A tile 1 tokens right behind tile 0 on the x queue
            if ua["nf"] > 1:
                ua["x1"] = xpool.tile([P, CB, TN], bf16, tag="xt", name="x1A")
                nc.sync.dma_start(ua["x1"][:, :, :], ua["xaps"][1][:, :, :])

            for u in units[1:]:
                slot = u["slot"]
                u["w1_sb"] = wpool.tile([P, CB, FQ], bf16, tag=f"w1{slot}",
                                        name=f"w1{slot}")
                nc.gpsimd.dma_start(u["w1_sb"][:, :, :], u["w1r"][:, :, :])
                u["w2_sb"] = wpool.tile([P, FBQ, D_MODEL], bf16, tag=f"w2{slot}",
                                        name=f"w2{slot}")
                nc.gpsimd.dma_start(u["w2_sb"][:, :, :], u["w2r"][:, :, :])

            for s, u in enumerate(units):
                u["b1o"], u["b2o"] = s * 16, s * 16 + 8
                if u["slot"] == "A":
                    u["w1ap"] = (lambda k, m, t=u["w1_sb"]:
                                 t[:, m // 2, k, ts(m % 2, P)])
                else:
                    u["w1ap"] = (lambda k, m, t=u["w1_sb"]:
                                 t[:, k, ts(m, P)])

            # ---- compute: per unit, bf16 token tiles then fp8 token tiles
            nu = len(units)
            for ui, u in enumerate(units):
                if u["cap8"]:
                    # fp8 weights land in a single shared buffer; the DMA
                    # WAR-waits on the previous unit's fp8 matmuls and runs
                    # during this unit's bf16 phase
                    u["w18_sb"] = w8pool.tile([P, CB, FQ], f8, tag="w18", name=f"w18{u['slot']}")
                    nc.gpsimd.dma_start(u["w18_sb"][:, :, :], u["w18r"][:, :, :])
                    u["w28_sb"] = w8pool.tile([P, FBQ, D_MODEL], f8, tag="w28", name=f"w28{u['slot']}")
                    nc.gpsimd.dma_start(u["w28_sb"][:, :, :], u["w28r"][:, :, :])

                for ti, tw in enumerate(u["tiles"]):
                    if ti == 0 and "x0" in u:
                        xt = u["x0"]
                    elif ti == 1 and "x1" in u:
                        xt = u["x1"]
                    else:
                        xt = xpool.tile([P, CB, TN], bf16, tag="xt")
                        nc.sync.dma_start(xt[:, :, :tw], u["xaps"][ti][:, :, :])

                    ht = hpool.tile([P, FBQ, TN], bf16, tag="ht")
                    for m in range(FBQ):
                        ph = psum.tile([P, TN], f32, tag="ph")
                        for k in range(CB):
                            nc.tensor.matmul(
                                ph[:, :tw],
                                lhsT=u["w1ap"](k, m),
                                rhs=xt[:, k, :tw],
                                start=(k == 0), stop=(k == CB - 1),
                            )
                        nc.scalar.activation(ht[:, m, :tw], ph[:, :tw], Act.Gelu,
                                             bias=bias_sb[:, u["b1o"] + m:u["b1o"] + m + 1])

                    yt = ypool.tile([P, CB, TN], bf16, tag="yt")
                    for c in range(CB):
                        py = psum.tile([P, TN], f32, tag="py")
                        for k in range(FBQ):
                            nc.tensor.matmul(
                                py[:, :tw],
                                lhsT=u["w2_sb"][:, k, ts(c, P)],
                                rhs=ht[:, k, :tw],
                                start=(k == 0), stop=(k == FBQ - 1),
                            )
                        nc.scalar.add(yt[:, c, :tw], py[:, :tw],
                                      bias_sb[:, u["b2o"] + c:u["b2o"] + c + 1])
                    nc.sync.dma_start(u["yaps"][ti][:, :, :], yt[:, :, :tw])

                for ti, tw in enumerate(u["tiles8"]):
                    x8t = x8pool.tile([P, CB, TN], f8, tag="x8")
                    nc.scalar.dma_start(x8t[:, :, :tw], u["x8aps"][ti][:, :, :])

                    h8t = h8pool.tile([P, FBQ, TN], f8, tag="h8")
                    for m in range(FBQ):
                        ph = psum.tile([P, TN], f32, tag="ph")
                        for j in range(CB // 2):
                            nc.tensor.matmul(
                                ph[:, :tw],
                                lhsT=u["w18_sb"][:, 2 * j:2 * j + 2, ts(m, P)],
                                rhs=x8t[:, 2 * j:2 * j + 2, :tw],
                                start=(j == 0), stop=(j == CB // 2 - 1),
                                perf_mode=DR,
                            )
                        # psum holds SW8 * z; gelu(psum/SW8 + b1) -> e4m3 h
                        nc.scalar.activation(h8t[:, m, :tw], ph[:, :tw], Act.Gelu,
                                             bias=bias_sb[:, u["b1o"] + m:u["b1o"] + m + 1],
                                             scale=1.0 / SW8)

                    last = (ui == nu - 1) and (ti == len(u["tiles8"]) - 1)
                    yt = ypool.tile([P, CB, TN], bf16, tag="yt")
                    for c in range(CB):
                        py = psum.tile([P, TN], f32, tag="py")
                        for j in range(FBQ // 2):
                            nc.tensor.matmul(
                                py[:, :tw],
                                lhsT=u["w28_sb"][:, 2 * j:2 * j + 2, ts(c, P)],
                                rhs=h8t[:, 2 * j:2 * j + 2, :tw],
                                start=(j == 0), stop=(j == FBQ // 2 - 1),
                                perf_mode=DR,
                            )
                        nc.scalar.activation(yt[:, c, :tw], py[:, :tw], Act.Identity,
                                             bias=bias_sb[:, u["b2o"] + c:u["b2o"] + c + 1],
                                             scale=1.0 / SW8)
                        if last:
                            # final tile: per-block stores overlap the tail
                            # GEMM2 blocks instead of one post-loop DMA
                            nc.sync.dma_start(u["y8aps"][ti][:, c, :tw],
                                              yt[:, c, :tw])
                    if not last:
                        nc.sync.dma_start(u["y8aps"][ti][:, :, :], yt[:, :, :tw])

    _fix_multiwait_bir(nc)
    _NC_CACHE[key] = nc
    return nc


def _route(xf, router_w, k):
    """Replicate the reference router numerics (f32 softmax, top-k, renorm)."""
    logits = xf @ router_w.T.astype(np.float32)          # [T, E]
    m = logits.max(axis=-1, keepdims=True)
    e = np.exp(logits - m, dtype=np.float32)
    probs = e / e.sum(axis=-1, keepdims=True)
    # descending, ties -> lower index first (matches jax.lax.top_k)
    idx = np.argsort(-probs, axis=-1, kind="stable")[:, :k]   # [T, k]
    w = np.take_along_axis(probs, idx, axis=-1)               # [T, k]
    w = w / (w.sum(axis=-1, keepdims=True) + 1e-9)
    return idx, w


def _align16(n):
    return max(P, -(-n // 16) * 16)


def _align16s(n):
    return max(16, -(-n // 16) * 16) if n else 0


def _pmaj(a2d, blocks):
    """[(g p), f] row-major 2D -> partition-major [P, blocks, f]."""
    return np.ascontiguousarray(
        a2d.reshape(blocks, P, a2d.shape[1]).transpose(1, 0, 2))


def _tile_tok(a2d, cap):
    """[C, n] token stream (n <= cap) -> list of partition-major per-tile
    arrays [P, CB, tw] (tw = TN for full tiles + remainder), zero-padded."""
    C = a2d.shape[0]
    pad = np.zeros((C, cap), dtype=a2d.dtype)
    pad[:, :a2d.shape[1]] = a2d
    tmp = pad.reshape(CB, P, cap)
    tiles, off = [], 0
    while off < cap:
        tw = min(TN, cap - off)
        tiles.append(np.ascontiguousarray(
            tmp[:, :, off:off + tw].transpose(1, 0, 2)))
        off += tw
    return tiles


def _untile_tok(tiles, cap):
    """inverse of _tile_tok -> [C, cap] f32."""
    parts = [t.astype(np.float32).transpose(1, 0, 2) for t in tiles]
    return np.concatenate(parts, axis=2).reshape(CB * P, cap)


def _q8(a, scale=1.0):
    a = np.asarray(a, np.float32) * np.float32(scale)
    np.clip(a, -240.0, 240.0, out=a)
    return a.astype(ml_dtypes.float8_e4m3)


def kernel(x, router_w, expert_w1, expert_b1, expert_w2, expert_b2, top_k):
    x = np.asarray(x)
    router_w = np.asarray(router_w, dtype=np.float32)
    expert_w1 = np.asarray(expert_w1, dtype=np.float32)
    expert_b1 = np.asarray(expert_b1, dtype=np.float32)
    expert_w2 = np.asarray(expert_w2, dtype=np.float32)
    expert_b2 = np.asarray(expert_b2, dtype=np.float32)
    k = int(np.asarray(top_k))
    Bq, Nq, C = x.shape
    Tq = Bq * Nq
    E = expert_w1.shape[0]
    xf = np.ascontiguousarray(x.reshape(Tq, C), dtype=np.float32)

    idx, w = _route(xf, router_w, k)

    # per expert: token list sorted by ascending combine weight, so the
    # lowest-weight FRAC8 go on the fp8 pipeline (their quantization noise
    # is scaled by w in the combine)
    tok8_idx, tok8_w, tokb_idx, tokb_w = [], [], [], []
    for e in range(E):
        mask = idx == e
        sel = np.nonzero(mask.any(axis=-1))[0]
        we = (w * mask).sum(axis=-1)[sel].astype(np.float32)
        order_w = np.argsort(we, kind="stable")
        n8 = int(round(FRAC8 * len(sel)))
        tok8_idx.append(sel[order_w[:n8]])
        tok8_w.append(we[order_w[:n8]])
        tokb_idx.append(sel[order_w[n8:]])
        tokb_w.append(we[order_w[n8:]])
    counts = np.array([len(s) for s in tokb_idx])
    counts8 = np.array([len(s) for s in tok8_idx])

    # slot s holds the quarters of the experts ranked 2s and 2s+1 by total
    # count; cores 0-3 take quarters 0-3 of the first, cores 4-7 the second
    order = np.argsort(-(counts + counts8), kind="stable")
    caps = [_align16(int(max(counts[order[2 * s]], counts[order[2 * s + 1]])))
            for s in range(4)]
    caps8 = [_align16s(int(max(counts8[order[2 * s]], counts8[order[2 * s + 1]])))
             for s in range(4)]

    nc = _build_moe_kernel(caps, caps8)

    # one xT (+xT8) per expert, shared by its four quarter-units
    xTs, xT8s = {}, {}
    for s in range(4):
        for j in (0, 1):
            e = int(order[2 * s + j])
            xTs[e] = _tile_tok(xf[tokb_idx[e]].T.astype(ml_dtypes.bfloat16),
                               caps[s])
            if caps8[s]:
                xT8s[e] = _tile_tok(_q8(xf[tok8_idx[e]].T), caps8[s])

    in_maps = [dict() for _ in range(N_CORES)]
    biasvs = [np.zeros((P, 16 * len(SLOTS)), dtype=np.float32)
              for _ in range(N_CORES)]
    placement = {}          # (expert, quarter) -> (core, slot name)
    for s, slot in enumerate(SLOTS):
        for core in range(N_CORES):
            e = int(order[2 * s + (0 if core < 4 else 1)])
            q = core % 4
            placement[(e, q)] = (core, slot)
            f0, f1 = q * FQ, (q + 1) * FQ
            b2 = expert_b2[e] if q == 0 else np.zeros(C, dtype=np.float32)
            w1q = np.ascontiguousarray(expert_w1[e, f0:f1].T)
            w2q = np.ascontiguousarray(expert_w2[e, :, f0:f1].T)
            bv = biasvs[core]
            bv[:, s * 16:s * 16 + 8] = expert_b1[e, f0:f1].reshape(8, P).T
            bv[:, s * 16 + 8:s * 16 + 16] = b2.reshape(8, P).T
            w1b = _pmaj(w1q.astype(ml_dtypes.bfloat16), CB)
            if slot == "A":
                for j in range(4):
                    in_maps[core][f"w1t{slot}_q{j}"] = np.ascontiguousarray(
                        w1b[:, :, j * (FQ // 4):(j + 1) * (FQ // 4)])
            else:
                in_maps[core][f"w1t{slot}"] = w1b
            in_maps[core][f"w2t{slot}"] = _pmaj(w2q.astype(ml_dtypes.bfloat16), FBQ)
            for i, tarr in enumerate(xTs[e]):
                in_maps[core][f"xT{slot}_{i}"] = tarr
            if caps8[s]:
                in_maps[core].update({
                    f"w1t8{slot}": _pmaj(_q8(w1q, SW8), CB),
                    f"w2t8{slot}": _pmaj(_q8(w2q, SW8), FBQ),
                })
                for i, tarr in enumerate(xT8s[e]):
                    in_maps[core][f"xT8{slot}_{i}"] = tarr

    for core in range(N_CORES):
        in_maps[core]["biasv"] = biasvs[core].reshape(-1)

    trace = os.environ.get("BASS_MOE_TRACE") == "1"
    res = run_bass_kernel_spmd(
        nc, in_maps, core_ids=list(range(N_CORES)),
        trace=trace,
        tmpdir=os.environ.get("BASS_MOE_TMPDIR") if trace else None,
    )
    if trace:
        kernel.last_exec_time_ns = res.exec_time_ns
        kernel.last_trace = (res.instructions_and_trace or (None, None))[1]

    out = np.zeros((Tq, C), dtype=np.float32)
    for e in range(E):
        for tok, tw_, pfx, cnt, capv in (
                (tokb_idx[e], tokb_w[e], "yT", counts[e], None),
                (tok8_idx[e], tok8_w[e], "yT8", counts8[e], None)):
            if not cnt:
                continue
            acc = np.zeros((cnt, C), dtype=np.float32)
            for q in range(4):
                core, slot = placement[(e, q)]
                s = SLOTS.index(slot)
                capv = caps[s] if pfx == "yT" else caps8[s]
                r = res.results[core]
                tlist = []
                i = 0
                while f"{pfx}{slot}_{i}" in r:
                    tlist.append(r[f"{pfx}{slot}_{i}"])
                    i += 1
                acc += _untile_tok(tlist, capv)[:, :cnt].T
            out[tok] += acc * tw_[:, None]
    return out.reshape(Bq, Nq, C).astype(x.dtype)
